# revision 50
# baseline (speedup 1.0000x reference)
"""BigBird sparse attention on 8 Trainium2 NeuronCores.

Sharding: 16 heads across 8 cores (2 heads/core, both batches per core).

Per-core pipeline (v3 — narrow bands, mask off the PE, rand on Pool+DVE):

  stage A  row-major fused QKV projection: per 128-row block, stationary =
           x-block [128dc,128r], moving = wqkv [128dc,384] -> psum [128,384],
           bias-add/copy into q-rows / kv-rows bf16 (DVE+Scalar).  KV row
           slices DMA directly to kv_stage DRAM.  x loads ride the Activation
           HWDGE queue; stores ride the SP queue.
  gathers  fused KV row gather (row-mode SWDGE, 1024B descriptors, 6 calls
           over 6 SWDGE queues), order (t, m, i) so chunk (t,m) holds
           key ri[t*128+i, m] at partition i.
  qT/kT    PE transposes (bf16 psum) + Scalar/DVE copies.
  QK       column layout scores^T[j,i]: stationary = kT j-block [64,128],
           moving = 384+2 contiguous q columns (3 i-blocks + global rows);
           j-block 0 streams all 2048 i columns.  exp (Scalar) reads psum
           directly; the BigBird keep-mask is applied AFTER exp as a binary
           multiply on DVE (exp(s+NEG*m) == exp(s)*keep since exp(NEG)~0).
  rand     random-key scores: big elementwise multiplies on GpSimd (Pool),
           free-axis reduces on DVE, tiny exp on Scalar.  Emitted early
           (right after gathers land) so wsum tiles are ready before each PV.
  PV       stationary = vaug j-block [128,65] (V rows + ones rider), moving =
           masked probability tiles; single [65,2048] psum accumulator per
           (b,h); row 64 = denominator.  Normalize: per 512-seg, den row ->
           SBUF (Scalar), fast-reciprocal of the [1,512] row (DVE), K=1
           broadcast matmul of the reciprocal row (PE), then DVE multiply
           ctx(psum) * bcast(psum) -> per-head hout.
  stage D  partial o-projection accumulating over the 2 heads (K=64);
           bf16 partial outputs summed on host.
"""

import math
import numpy as np

# ---------------------------------------------------------------- constants
B = 2
S = 2048
D = 1024
H = 16
HD = 64
NG = 2          # num global tokens
NR = 3          # num random keys per row
WIN = 3         # window half-width

N_CORES = 8
HPC = H // N_CORES          # heads per core = 2
HD2 = HPC * HD              # 128 = head-dim slice per core
R = B * S                   # 4096 flattened rows
NT = S // 128               # 16 i-blocks per (b, h)
NRB = 32                    # row blocks (R / 128)
NCH = NT * NR               # 48 gather chunks of 128
NIDX = NCH * 128            # 6144 gather indices
KVROW = 512                 # kv_stage row elements (2 batches x (K|V) x 128)

INV_SQRT_HD = 1.0 / math.sqrt(float(HD))


# i-column range served by k-block jb (band + global rows); jb 0 serves all.
# For jb >= 1 the window |i-j|<=3 with j in [128jb, 128jb+128) only needs
# i in [128jb-3, 128jb+131) — 134 columns, not 384.
BW = 134


def _serve(jb):
    if jb == 0:
        return 0, 2048, False
    i0 = 128 * jb - 3
    w = min(BW, 2048 - i0)
    return i0, w, True   # extra 2 cols for global rows


def _mask_kind(jb):
    if jb == 0:
        return 0
    if jb == 15:
        return 3
    return 2


# ---------------------------------------------------------------- host prep
def _host_masks():
    """Binary KEEP masks for the structural (window|global) pattern, [j, i]."""
    import ml_dtypes

    j = np.arange(S)[:, None]
    i = np.arange(S)[None, :]
    Dm = (i < NG) | (j < NG) | (np.abs(i - j) <= WIN)   # [j, i] structural

    c0 = Dm[0:128, :].astype(np.float32)                          # [128,2048]
    # band tile for jb>=1: i = 128jb-3+ic, j = 128jb+jr -> keep iff
    # |ic-3-jr| <= 3 iff jr <= ic <= jr+6
    jr = np.arange(128)[:, None]
    ic = np.arange(BW)[None, :]
    cI = ((ic >= jr) & (ic <= jr + 6)).astype(np.float32)
    cI = np.concatenate([cI, np.ones((128, 2), np.float32)],
                        axis=1)                                    # [128, 136]
    c15 = np.concatenate([cI[:, 0:131], np.ones((128, 2), np.float32)],
                         axis=1)                                   # [128, 133]
    # 3-band concatenated masks (bands are processed in groups of three
    # sharing one psum tile / one exp / one mask multiply)
    cg = np.concatenate([cI, cI, cI], axis=1)                      # [128, 408]
    cg15 = np.concatenate([cI, cI, c15], axis=1)                   # [128, 405]
    bf = ml_dtypes.bfloat16
    return c0.astype(bf), cg.astype(bf), cg15.astype(bf)


def _host_idx_w(random_indices):
    """Gather indices (t, m, i order) and dedup weights w [128, 48]."""
    import ml_dtypes

    ri = np.asarray(random_indices).astype(np.int64)   # [S, NR]
    n = np.arange(NIDX)
    t_of = n // (NR * 128)
    m_of = (n // 128) % NR
    p_of = n % 128
    i_of = t_of * 128 + p_of
    j_of = ri[i_of, m_of]
    a16 = np.zeros((16, NIDX // 16), dtype=np.int16)
    a16[n % 16, n // 16] = j_of.astype(np.int16)
    gidx = np.tile(a16, (8, 1))                        # [128, NIDX//16]

    # w[p, (t, m)]: 0 if structurally covered / global row / duplicate
    w = np.ones((128, NCH), dtype=np.float32)
    for t in range(NT):
        for m in range(NR):
            c = t * NR + m
            for p in range(128):
                i = t * 128 + p
                r = ri[i, m]
                if i < NG or r < NG or abs(i - r) <= WIN:
                    w[p, c] = 0.0
                elif any(ri[i, mm] == r for mm in range(m)):
                    w[p, c] = 0.0
    return gidx, w.astype(ml_dtypes.bfloat16)


def make_in_maps(inputs):
    """Full inputs -> list of 8 per-core input dicts."""
    import ml_dtypes
    bf = ml_dtypes.bfloat16

    x = np.asarray(inputs["x"], dtype=np.float32)
    ri = np.asarray(inputs["random_indices"])
    q_w = np.asarray(inputs["q_w"], dtype=np.float32) * INV_SQRT_HD
    k_w = np.asarray(inputs["k_w"], dtype=np.float32)
    v_w = np.asarray(inputs["v_w"], dtype=np.float32)
    o_w = np.asarray(inputs["o_w"], dtype=np.float32)
    q_b = np.asarray(inputs["q_b"], dtype=np.float32) * INV_SQRT_HD
    k_b = np.asarray(inputs["k_b"], dtype=np.float32)
    v_b = np.asarray(inputs["v_b"], dtype=np.float32)

    # x blocks: xprep[p, rb, dc, rl] = xT[dc*128+p, rb*128+rl]
    xT = np.ascontiguousarray(x.reshape(R, D).T)                  # [D, R]
    xprep = np.ascontiguousarray(
        xT.reshape(8, 128, NRB, 128).transpose(1, 2, 0, 3)).astype(bf)

    mc0, mcI, mc15 = _host_masks()
    gidx, wrand = _host_idx_w(ri)

    in_maps = []
    for c in range(N_CORES):
        sl = slice(HD2 * c, HD2 * (c + 1))
        wqkv = np.concatenate(
            [q_w[sl, :].T, k_w[sl, :].T, v_w[sl, :].T], axis=1)   # [D, 384]
        wqkv = np.ascontiguousarray(wqkv.reshape(8, 128, 384)).astype(bf)
        bias = np.concatenate([q_b[sl], k_b[sl], v_b[sl]])        # [384]
        biasb = np.ascontiguousarray(
            np.broadcast_to(bias[None, :], (128, 384))).copy()
        in_maps.append({
            "xprep": xprep,
            "wqkv": wqkv,
            "biasb": biasb,
            "mc0": mc0, "mcI": mcI, "mc15": mc15,
            "wrand": wrand,
            "gidx": gidx,
            "wo2": np.ascontiguousarray(o_w[:, sl].T).astype(bf),  # [128, D]
        })
    return in_maps


# ---------------------------------------------------------------- device IR
def build_kernel(bias_zero=True):
    import os
    import concourse.tile as tile
    from concourse import bacc, mybir

    nc = bacc.Bacc("TRN2", target_bir_lowering=False, debug=False,
                   num_swdge_queues=int(os.environ.get("K_GQ", "4")))
    f32 = mybir.dt.float32
    bf16 = mybir.dt.bfloat16
    i16 = mybir.dt.int16

    t_ = dict(
        xprep=nc.dram_tensor("xprep", [128, NRB, 8, 128], bf16,
                             kind="ExternalInput").ap(),
        wqkv=nc.dram_tensor("wqkv", [8, 128, 384], bf16,
                            kind="ExternalInput").ap(),
        biasb=nc.dram_tensor("biasb", [128, 384], f32,
                             kind="ExternalInput").ap(),
        mc0=nc.dram_tensor("mc0", [128, 2048], bf16,
                           kind="ExternalInput").ap(),
        mcI=nc.dram_tensor("mcI", [128, 408], bf16,
                           kind="ExternalInput").ap(),
        mc15=nc.dram_tensor("mc15", [128, 405], bf16,
                            kind="ExternalInput").ap(),
        wrand=nc.dram_tensor("wrand", [128, NCH], bf16,
                             kind="ExternalInput").ap(),
        gidx=nc.dram_tensor("gidx", [128, NIDX // 16], i16,
                            kind="ExternalInput").ap(),
        wo2=nc.dram_tensor("wo2", [HD2, D], bf16, kind="ExternalInput").ap(),
        out=nc.dram_tensor("out_part", [R, D], bf16,
                           kind="ExternalOutput").ap(),
        kv_stage=nc.dram_tensor("kv_stage", [S, KVROW], bf16).ap(),
    )
    if os.environ.get("K_DEBUG", "0") == "1":
        for nm, shp in (("dbg_qrows", [128, NRB, 128]),
                        ("dbg_kvrows", [128, NRB, 256]),
                        ("dbg_qT", [128, R]), ("dbg_kT", [128, R]),
                        ("dbg_ksel0", [128, NCH, 256]),
                        ("dbg_ksel1", [128, NCH, 256]),
                        ("dbg_pj", [128, 4096]),
                        ("dbg_wsumf", [128, NT, HD + 1]),
                        ("dbg_hout0", [HD, R]), ("dbg_hout1", [HD, R])):
            dt = bf16
            t_[nm] = nc.dram_tensor(nm, shp, dt, kind="ExternalOutput").ap()

    with tile.TileContext(nc) as tc:
        _build_tc(nc, tc, t_, bias_zero)
    nc.compile()
    return nc


def _build_tc(nc, tc, t_, bias_zero):
    import os
    from contextlib import ExitStack

    import concourse.bass as bass
    from concourse import masks as cmasks, mybir

    GQ = int(os.environ.get("K_GQ", "4"))
    XQ_ACT = os.environ.get("K_XQ", "act") == "act"
    NORM2P = os.environ.get("K_NORM2P", "1") == "1"   # ctx*bc both-psum mult
    DEBUG = os.environ.get("K_DEBUG", "0") == "1"

    def dbg(name, src_ap):
        if DEBUG and name in t_:
            nc.sync.dma_start(t_[name], src_ap)

    f32 = mybir.dt.float32
    bf16 = mybir.dt.bfloat16
    EXP = mybir.ActivationFunctionType.Exp
    COPY = mybir.ActivationFunctionType.Copy
    MULT = mybir.AluOpType.mult
    ADD = mybir.AluOpType.add
    X = mybir.AxisListType.X
    ts = bass.ts

    with ExitStack() as ctx:
        const = ctx.enter_context(tc.tile_pool(name="const", bufs=1))
        persist = ctx.enter_context(tc.tile_pool(name="persist", bufs=1))

        # ---- constants, spread over HWDGE queues so wqkv lands first
        ident = const.tile([128, 128], bf16)
        cmasks.make_identity(nc, ident[:])
        ones_t = const.tile([HD + 1, HD], bf16)
        nc.vector.memset(ones_t[:], 1.0)

        # wqkv first on the SP queue, split per dc so the first stage A
        # matmul only waits for its own contraction slice
        wqkv_sb = const.tile([128, 8, 384], bf16)
        for dc in range(8):
            nc.sync.dma_start(wqkv_sb[:, dc, :], t_["wqkv"][dc, :, :])
        biasb_sb = None
        if not bias_zero:
            biasb_sb = const.tile([128, 384], f32)
            nc.sync.dma_start(biasb_sb[:], t_["biasb"])
        # gather tables ride the Pool queue (ahead of the SWDGE gathers)
        gidx_sb = const.tile([128, NIDX // 16], mybir.dt.int16)
        nc.gpsimd.dma_start(gidx_sb[:], t_["gidx"])
        wrand_sb = const.tile([128, NCH], bf16)
        nc.gpsimd.dma_start(wrand_sb[:], t_["wrand"])
        mask_sb = {
            0: const.tile([128, 2048], bf16, name="mc0"),
            2: const.tile([128, 408], bf16, name="mcI"),
            3: const.tile([128, 405], bf16, name="mc15"),
        }
        for k, nm in ((0, "mc0"), (2, "mcI"), (3, "mc15")):
            nc.sync.dma_start(mask_sb[k][:], t_[nm])
        wo2_sb = const.tile([HD2, D], bf16)
        nc.sync.dma_start(wo2_sb[:], t_["wo2"])

        # ---- persistent activations
        qrows = persist.tile([128, NRB, 128], bf16)        # row-major q
        qT = persist.tile([128, R], bf16)
        kT = persist.tile([128, R], bf16)
        vaug = [[persist.tile([128, NT, HD + 1], bf16, name=f"vaug{b}{h}")
                 for h in range(HPC)] for b in range(B)]
        # h0's context lives in hout2[0:64]; h1 normalizes into a base-0
        # scratch then DMA-copies across partitions into hout2[64:128]
        hout2 = persist.tile([128, R], bf16)
        hout1 = persist.tile([HD, R], bf16)
        kselp = ctx.enter_context(tc.tile_pool(name="kselp", bufs=1))
        ksel = kselp.tile([128, NCH, KVROW], bf16)         # gathered KV rows

        # ---- stage A: fused row-major QKV projection (pairs of row blocks)
        with tc.tile_pool(name="kvrows", bufs=1) as kvpool:
            kvrows = kvpool.tile([128, NRB, 256], bf16)    # row-major k|v
            with tc.tile_pool(name="xstream", bufs=6) as xpool, \
                    tc.tile_pool(name="apsum", bufs=3, space="PSUM") as apsum:
                for pr in range(NRB // 2):
                    xt = xpool.tile([128, 2, 8, 128], bf16, tag="xt")
                    xeng = nc.scalar if XQ_ACT else nc.sync
                    if pr == 0:
                        xeng = nc.gpsimd   # SWDGE queue starts ~8us faster
                    xeng.dma_start(
                        xt[:], t_["xprep"][:, 2 * pr:2 * pr + 2, :, :])
                    # pad to 512/f32 so each j's output stays in one bank
                    ps = apsum.tile([128, 2, 512], f32, tag="ps")
                    for j in range(2):
                        for dc in range(8):
                            nc.tensor.matmul(ps[:, j, 0:384],
                                             xt[:, j, dc, :],
                                             wqkv_sb[:, dc, :],
                                             start=(dc == 0), stop=(dc == 7))
                    rb = 2 * pr
                    if bias_zero:
                        if pr % 2 == 0:
                            nc.scalar.activation(qrows[:, rb:rb + 2, :],
                                                 ps[:, :, 0:128], COPY)
                            nc.scalar.activation(kvrows[:, rb:rb + 2, :],
                                                 ps[:, :, 128:384], COPY)
                        else:
                            nc.vector.tensor_copy(qrows[:, rb:rb + 2, :],
                                                  ps[:, :, 0:128])
                            nc.vector.tensor_copy(kvrows[:, rb:rb + 2, :],
                                                  ps[:, :, 128:384])
                    else:
                        nc.vector.tensor_tensor(
                            qrows[:, rb:rb + 2, :], ps[:, :, 0:128],
                            biasb_sb[:, 0:128].unsqueeze(1)
                            .broadcast_to([128, 2, 128]), op=ADD)
                        nc.vector.tensor_tensor(
                            kvrows[:, rb:rb + 2, :], ps[:, :, 128:384],
                            biasb_sb[:, 128:384].unsqueeze(1)
                            .broadcast_to([128, 2, 256]), op=ADD)
                    b = rb // NT
                    r0 = (rb % NT) * 128
                    nc.sync.dma_start(
                        t_["kv_stage"][r0:r0 + 256, b * 256:(b + 1) * 256]
                        .rearrange("(r p) c -> p r c", p=128),
                        kvrows[:, rb:rb + 2, :])

            # ---- gathers: 6 row-mode SWDGE calls across GQ queues
            for u in range(NIDX // 1024):
                nc.gpsimd.dma_gather(
                    ksel[:, u * 8:(u + 1) * 8, :], t_["kv_stage"],
                    gidx_sb[:, u * 64:(u + 1) * 64],
                    1024, 1024, KVROW, transpose=False, queue_num=u % GQ)

            # ---- qT / kT transposes
            with tc.tile_pool(name="tpsum", bufs=4, space="PSUM") as tpsum:
                for rb in range(NRB):
                    ptq = tpsum.tile([128, 128], bf16, tag="tp")
                    nc.tensor.transpose(ptq[:], qrows[:, rb, :], ident[:])
                    ptk = tpsum.tile([128, 128], bf16, tag="tp")
                    nc.tensor.transpose(ptk[:], kvrows[:, rb, 0:128],
                                        ident[:])
                    if rb % 2 == 0:
                        nc.scalar.activation(qT[:, ts(rb, 128)], ptq[:],
                                             COPY)
                        nc.vector.tensor_copy(kT[:, ts(rb, 128)], ptk[:])
                    else:
                        nc.vector.tensor_copy(qT[:, ts(rb, 128)], ptq[:])
                        nc.scalar.activation(kT[:, ts(rb, 128)], ptk[:],
                                             COPY)

            # ---- vaug: V rows + ones rider
            for b in range(B):
                for h in range(HPC):
                    nc.vector.tensor_copy(
                        vaug[b][h][:, :, 0:HD],
                        kvrows[:, b * NT:(b + 1) * NT,
                               128 + HD * h:128 + HD * (h + 1)])
                    nc.vector.memset(vaug[b][h][:, :, HD:HD + 1], 1.0)

            dbg("dbg_qrows", qrows[:])
            dbg("dbg_kvrows", kvrows[:])
            dbg("dbg_qT", qT[:])
            dbg("dbg_kT", kT[:])
            dbg("dbg_ksel0", ksel[:, :, 0:256])
            dbg("dbg_ksel1", ksel[:, :, 256:512])

        # ---- attention
        with tc.tile_pool(name="pj", bufs=4) as pjpool, \
                tc.tile_pool(name="rbig", bufs=4) as rbig, \
                tc.tile_pool(name="rsm", bufs=4) as rsm, \
                tc.tile_pool(name="wsb", bufs=4) as wpool, \
                tc.tile_pool(name="dn2", bufs=2) as dn2, \
                tc.tile_pool(name="dn4", bufs=4) as dn4, \
                tc.tile_pool(name="osb", bufs=3) as opool, \
                tc.tile_pool(name="ctx", bufs=1, space="PSUM") as ctxpsum, \
                tc.tile_pool(name="s512", bufs=3, space="PSUM") as spsum, \
                tc.tile_pool(name="bc", bufs=1, space="PSUM") as bcpsum:

            pj_t = {}
            coef_t = {}
            wsumb_t = {}
            deferred_masks = []

            def qk_phase(b, h, mask_now=True):
                hs = slice(HD * h, HD * (h + 1))
                pj = pjpool.tile([128, 4096], bf16, tag="pj",
                                 name=f"pj{b}{h}")
                pj_t[(b, h)] = pj

                def mask_mult(lo, tot, kind, seg=None):
                    msk = (mask_sb[0][:, ts(seg, 512)] if kind == 0
                           else mask_sb[kind][:, 0:tot])
                    op = (nc.vector.tensor_tensor,
                          pj[:, lo:lo + tot], msk)
                    if mask_now:
                        op[0](op[1], op[1], op[2], op=MULT)
                    else:
                        deferred_masks.append(op)

                # j-block 0 serves every i column
                for seg in range(4):
                    psc = spsum.tile([128, 512], f32, tag="psc")
                    nc.tensor.matmul(
                        psc[:], kT[hs, b * S:b * S + 128],
                        qT[hs, b * S + seg * 512:b * S + (seg + 1) * 512],
                        start=True, stop=True)
                    nc.scalar.activation(
                        pj[:, seg * 512:(seg + 1) * 512], psc[:], EXP)
                    mask_mult(seg * 512, 512, 0, seg)
                col = 2048
                # bands in groups of three sharing one psum tile, one exp
                # and one mask multiply
                for g in range(5):
                    psc = spsum.tile([128, 512], f32, tag="psc")
                    off = 0
                    for jb in range(3 * g + 1, 3 * g + 4):
                        i0, w, extra = _serve(jb)
                        lhsT = kT[hs, b * S + jb * 128:
                                  b * S + (jb + 1) * 128]
                        nc.tensor.matmul(psc[:, off:off + w], lhsT,
                                         qT[hs, b * S + i0:b * S + i0 + w],
                                         start=(jb == 3 * g + 1), stop=False)
                        # start=False: the group's first matmul bank-clear
                        # already reset has_written for this psum tile
                        nc.tensor.matmul(
                            psc[:, off + w:off + w + 2], lhsT,
                            qT[hs, b * S:b * S + 2],
                            start=False, stop=(jb == 3 * g + 3))
                        off += w + 2
                    nc.scalar.activation(pj[:, col:col + off],
                                         psc[:, 0:off], EXP)
                    mask_mult(col, off, 3 if g == 4 else 2)
                    col += off
                if (b, h) == (0, 0):
                    dbg("dbg_pj", pj[:])

            def flush_masks():
                for fn, ap, msk in deferred_masks:
                    fn(ap, ap, msk, op=MULT)
                deferred_masks.clear()

            def rand_a(b, h, eng):
                # random-key scores: multiply on eng, DVE reduce, Scalar exp
                kv_k = ksel[:, :, b * 256 + HD * h:b * 256 + HD * (h + 1)]
                kv_k = kv_k.rearrange("p (t m) c -> p t m c", t=NT)
                qrep = qrows[:, b * NT:(b + 1) * NT, HD * h:HD * (h + 1)]
                qrep = qrep.unsqueeze(2).broadcast_to([128, NT, NR, HD])
                dotp = rbig.tile([128, NT, NR, HD], bf16, tag="big",
                                 name=f"dotp{b}{h}")
                eng.tensor_tensor(dotp[:], kv_k, qrep, op=MULT)
                sval = rsm.tile([128, NCH], f32, tag="sval")
                nc.vector.tensor_reduce(
                    sval[:].rearrange("p (t m) -> p t m", t=NT), dotp[:],
                    axis=X, op=ADD)
                coef = rsm.tile([128, NCH], bf16, tag="coef",
                                name=f"coef{b}{h}")
                coef_t[(b, h)] = coef
                nc.scalar.activation(coef[:], sval[:], EXP)
                nc.vector.tensor_tensor(coef[:], coef[:], wrand_sb[:],
                                        op=MULT)

            def rand_b(b, h, eng):
                # weighted V rows: multiply on eng, DVE reduces straight to
                # bf16 (summing <=3 O(1) terms — fp32 accumulate unneeded)
                coef = coef_t[(b, h)]
                kv_v = ksel[:, :, b * 256 + 128 + HD * h:
                            b * 256 + 128 + HD * (h + 1)]
                kv_v = kv_v.rearrange("p (t m) c -> p t m c", t=NT)
                crep = coef[:].rearrange("p (t m) -> p t m", t=NT)
                crep = crep.unsqueeze(3).broadcast_to([128, NT, NR, HD])
                dotv = rbig.tile([128, NT, NR, HD], bf16, tag="big",
                                 name=f"dotv{b}{h}")
                eng.tensor_tensor(dotv[:], kv_v, crep, op=MULT)
                wsumb = wpool.tile([128, NT, HD + 1], bf16, tag="wsumb",
                                   name=f"wsumb{b}{h}")
                wsumb_t[(b, h)] = wsumb
                with nc.allow_low_precision("3-term O(1) rand sums"):
                    nc.vector.tensor_reduce(
                        wsumb[:, :, 0:HD],
                        dotv[:].rearrange("p t m c -> p t c m"),
                        axis=X, op=ADD)
                    nc.vector.tensor_reduce(
                        wsumb[:, :, HD:HD + 1],
                        coef[:].rearrange("p (t m) -> p t m", t=NT),
                        axis=X, op=ADD)
                if (b, h) == (0, 0):
                    dbg("dbg_wsumf", wsumb[:])

            def pv_phase(b, h):
                pj = pj_t[(b, h)]
                wsumb = wsumb_t[(b, h)]
                ctxp = ctxpsum.tile([128, 2048], f32, tag="ctx")
                col = 0
                for jb in range(NT):
                    i0, w, extra = _serve(jb)
                    lhsT = vaug[b][h][:, jb, :]
                    if jb == 0:
                        for seg in range(4):
                            nc.tensor.matmul(
                                ctxp[0:HD + 1, ts(seg, 512)], lhsT,
                                pj[:, ts(seg, 512)], start=True, stop=False)
                        col = 2048
                        continue
                    # split at 512-col PSUM bank boundaries
                    a = i0
                    while a < i0 + w:
                        e = min(i0 + w, (a // 512 + 1) * 512)
                        nc.tensor.matmul(
                            ctxp[0:HD + 1, a:e], lhsT,
                            pj[:, col + (a - i0):col + (e - i0)],
                            start=False, stop=False)
                        a = e
                    if extra:
                        nc.tensor.matmul(ctxp[0:HD + 1, 0:2], lhsT,
                                         pj[:, col + w:col + w + 2],
                                         start=False, stop=False)
                    col += w + (2 if extra else 0)
                # wsum merges per 512-seg; den copy as soon as a seg's
                # columns are final, so the bcast matmul never waits
                dens = []
                for sgi in range(4):
                    for t in range(4 * sgi, 4 * sgi + 4):
                        nc.tensor.matmul(ctxp[0:HD + 1, ts(t, 128)],
                                         wsumb[:, t, :], ident[:],
                                         start=False, stop=True)
                    den = dn4.tile([HD + 1, 512], bf16, tag="den")
                    nc.scalar.activation(den[HD:HD + 1, :],
                                         ctxp[HD:HD + 1, ts(sgi, 512)], COPY)
                    dens.append(den)
                dst = hout2[0:HD, :] if h == 0 else hout1[:]
                for sgi in range(4):
                    bcp = bcpsum.tile([HD, 512], f32, tag="bc")
                    nc.tensor.matmul(bcp[:], ones_t[HD:HD + 1, :],
                                     dens[sgi][HD:HD + 1, :],
                                     start=True, stop=True)
                    # approx-fast reciprocal misreads PSUM sources: stage
                    # via SBUF (GPSIMD cannot read PSUM, so Scalar does it)
                    bsb = dn2.tile([HD, 512], f32, tag="bsb")
                    nc.scalar.activation(bsb[:], bcp[:], COPY)
                    rinv = dn2.tile([HD, 512], f32, tag="rinv")
                    nc.vector.reciprocal_approx_fast(rinv[:], bsb[:])
                    nc.vector.tensor_tensor(
                        dst[:, b * S + sgi * 512:b * S + (sgi + 1) * 512],
                        ctxp[0:HD, ts(sgi, 512)], rinv[:], op=MULT)
                if h == 1:
                    # cross-partition merge (only DMA can change partitions)
                    nc.sync.dma_start(
                        hout2[HD:HD2, b * S:(b + 1) * S],
                        hout1[:, b * S:(b + 1) * S])

            def d_stage(b, scalar_only=False):
                # partial o-projection for batch b (interleaves attention)
                for rc in range(b * NT, (b + 1) * NT):
                    ob = opool.tile([128, D], bf16, tag="ob")
                    for half in range(2):
                        po = spsum.tile([128, 512], f32, tag="psc")
                        nc.tensor.matmul(po[:], hout2[:, ts(rc, 128)],
                                         wo2_sb[:, ts(half, 512)],
                                         start=True, stop=True)
                        if not scalar_only and (rc * 2 + half) % 2 == 0:
                            nc.vector.tensor_copy(ob[:, ts(half, 512)], po[:])
                        else:
                            nc.scalar.activation(ob[:, ts(half, 512)], po[:],
                                                 COPY)
                    # alternate HWDGE queues for the output stream
                    oeng = nc.sync if rc % 2 == 0 else nc.scalar
                    oeng.dma_start(t_["out"][ts(rc, 128), :], ob[:])

            # software pipeline: all four QK phases first (pj bufs=4), then
            # PVs in order.  Pair (0,0)'s whole rand chain runs on DVE
            # (lowest latency after the gathers land); the other pairs'
            # multiplies go to Pool.  Masks for the second batch are
            # deferred past pair (0,0)'s chain so the DVE FIFO serves the
            # critical chain first.
            qk_phase(0, 0)
            qk_phase(0, 1)
            qk_phase(1, 0, mask_now=False)
            qk_phase(1, 1, mask_now=False)
            rand_a(0, 0, nc.vector)
            rand_b(0, 0, nc.gpsimd)
            flush_masks()
            pv_phase(0, 0)
            rand_a(0, 1, nc.gpsimd)
            rand_a(1, 0, nc.gpsimd)
            rand_a(1, 1, nc.gpsimd)
            rand_b(0, 1, nc.gpsimd)
            pv_phase(0, 1)
            rand_b(1, 0, nc.gpsimd)
            d_stage(0, scalar_only=True)
            rand_b(1, 1, nc.gpsimd)
            pv_phase(1, 0)
            pv_phase(1, 1)
            dbg("dbg_hout0", hout2[0:HD, :])
            dbg("dbg_hout1", hout1[:])

        # ---- stage D batch 1 (tail): wide psum tiles, single wide copies
        with tc.tile_pool(name="osb2", bufs=4) as opool2, \
                tc.tile_pool(name="opsum", bufs=2, space="PSUM") as opsum:
            for rc in range(NT, NRB):
                ob = opool2.tile([128, D], bf16, tag="ob")
                po = opsum.tile([128, 2, 512], f32, tag="po")
                for half in range(2):
                    nc.tensor.matmul(po[:, half, :], hout2[:, ts(rc, 128)],
                                     wo2_sb[:, ts(half, 512)],
                                     start=True, stop=True)
                if rc % 2 == 0:
                    nc.vector.tensor_copy(ob[:], po[:])
                else:
                    nc.scalar.activation(ob[:], po[:], COPY)
                oeng = nc.sync if rc % 2 == 0 else nc.scalar
                oeng.dma_start(t_["out"][ts(rc, 128), :], ob[:])


# ---------------------------------------------------------------- execution
_NC_CACHE = {}


def _get_nc(bias_zero=True):
    if bias_zero not in _NC_CACHE:
        _NC_CACHE[bias_zero] = build_kernel(bias_zero)
    return _NC_CACHE[bias_zero]


def _install_axon_trace_shim():
    import sys
    import types

    if "antenv.axon_hooks" in sys.modules:
        return
    mod = types.ModuleType("antenv.axon_hooks")
    mod._hook = None
    mod.set_axon_ntff_profile_hook = lambda h: setattr(mod, "_hook", h)
    mod.get_axon_ntff_profile_hook = lambda: mod._hook
    sys.modules["antenv.axon_hooks"] = mod
    try:
        import antenv
        antenv.axon_hooks = mod
        from trn_agent_boot.trn_boot import _ntff_profile_via_ctypes
        mod._hook = _ntff_profile_via_ctypes("/opt/axon/libaxon_pjrt.so")
    except Exception:
        pass


def run_on_hw(in_maps, trace=False, trace_kwargs=None, bias_zero=True):
    """Compile+run on the 8 cores; returns (results, BassKernelResults)."""
    _install_axon_trace_shim()
    from concourse import bass_utils
    bass_utils.upload_artifacts = lambda tmpdir: f"local:{tmpdir}"

    nc = _get_nc(bias_zero)
    res = bass_utils.run_bass_kernel_spmd(
        nc, in_maps, core_ids=list(range(N_CORES)), trace=trace,
        trace_kwargs=trace_kwargs or {})
    return res.results, res


def _bias_zero(inputs):
    return all(
        not np.any(np.asarray(inputs[k], dtype=np.float32))
        for k in ("q_b", "k_b", "v_b"))


def kernel(**inputs):
    in_maps = make_in_maps(inputs)
    results, _ = run_on_hw(in_maps, trace=False, bias_zero=_bias_zero(inputs))
    out = np.zeros((R, D), dtype=np.float32)
    for c in range(N_CORES):
        out += np.asarray(results[c]["out_part"], dtype=np.float32)
    out += np.asarray(inputs["o_b"], dtype=np.float32)[None, :]
    return out.reshape(B, S, D)


# revision 51
# speedup vs baseline: 1.1819x; 1.1819x over previous
"""BigBird sparse attention on 8 Trainium2 NeuronCores.

Sharding: 16 heads across 8 cores (2 heads/core, both batches per core).

Per-core pipeline (v3 — narrow bands, mask off the PE, rand on Pool+DVE):

  stage A  row-major fused QKV projection: per 128-row block, stationary =
           x-block [128dc,128r], moving = wqkv [128dc,384] -> psum [128,384],
           bias-add/copy into q-rows / kv-rows bf16 (DVE+Scalar).  KV row
           slices DMA directly to kv_stage DRAM.  x loads ride the Activation
           HWDGE queue; stores ride the SP queue.
  gathers  fused KV row gather (row-mode SWDGE, 1024B descriptors, 6 calls
           over 6 SWDGE queues), order (t, m, i) so chunk (t,m) holds
           key ri[t*128+i, m] at partition i.
  qT/kT    PE transposes (bf16 psum) + Scalar/DVE copies.
  QK       column layout scores^T[j,i]: stationary = kT j-block [64,128],
           moving = 384+2 contiguous q columns (3 i-blocks + global rows);
           j-block 0 streams all 2048 i columns.  exp (Scalar) reads psum
           directly; the BigBird keep-mask is applied AFTER exp as a binary
           multiply on DVE (exp(s+NEG*m) == exp(s)*keep since exp(NEG)~0).
  rand     random-key scores: big elementwise multiplies on GpSimd (Pool),
           free-axis reduces on DVE, tiny exp on Scalar.  Emitted early
           (right after gathers land) so wsum tiles are ready before each PV.
  PV       stationary = vaug j-block [128,65] (V rows + ones rider), moving =
           masked probability tiles; single [65,2048] psum accumulator per
           (b,h); row 64 = denominator.  Normalize: per 512-seg, den row ->
           SBUF (Scalar), fast-reciprocal of the [1,512] row (DVE), K=1
           broadcast matmul of the reciprocal row (PE), then DVE multiply
           ctx(psum) * bcast(psum) -> per-head hout.
  stage D  partial o-projection accumulating over the 2 heads (K=64);
           bf16 partial outputs summed on host.
"""

import math
import numpy as np

# ---------------------------------------------------------------- constants
B = 2
S = 2048
D = 1024
H = 16
HD = 64
NG = 2          # num global tokens
NR = 3          # num random keys per row
WIN = 3         # window half-width

N_CORES = 8
HPC = H // N_CORES          # heads per core = 2
HD2 = HPC * HD              # 128 = head-dim slice per core
R = B * S                   # 4096 flattened rows
NT = S // 128               # 16 i-blocks per (b, h)
NRB = 32                    # row blocks (R / 128)
NCH = NT * NR               # 48 gather chunks of 128
NIDX = NCH * 128            # 6144 gather indices
KVROW = 512                 # kv_stage row elements (2 batches x (K|V) x 128)

INV_SQRT_HD = 1.0 / math.sqrt(float(HD))


# i-column range served by k-block jb (band + global rows); jb 0 serves all.
# For jb >= 1 the window |i-j|<=3 with j in [128jb, 128jb+128) only needs
# i in [128jb-3, 128jb+131) — 134 columns, not 384.
BW = 134


def _serve(jb):
    if jb == 0:
        return 0, 2048, False
    i0 = 128 * jb - 3
    w = min(BW, 2048 - i0)
    return i0, w, True   # extra 2 cols for global rows


def _mask_kind(jb):
    if jb == 0:
        return 0
    if jb == 15:
        return 3
    return 2


# ---------------------------------------------------------------- host prep
def _host_masks():
    """Binary KEEP masks for the structural (window|global) pattern, [j, i]."""
    import ml_dtypes

    j = np.arange(S)[:, None]
    i = np.arange(S)[None, :]
    Dm = (i < NG) | (j < NG) | (np.abs(i - j) <= WIN)   # [j, i] structural

    c0 = Dm[0:128, :].astype(np.float32)                          # [128,2048]
    # band tile for jb>=1: i = 128jb-3+ic, j = 128jb+jr -> keep iff
    # |ic-3-jr| <= 3 iff jr <= ic <= jr+6
    jr = np.arange(128)[:, None]
    ic = np.arange(BW)[None, :]
    cI = ((ic >= jr) & (ic <= jr + 6)).astype(np.float32)
    cI = np.concatenate([cI, np.ones((128, 2), np.float32)],
                        axis=1)                                    # [128, 136]
    c15 = np.concatenate([cI[:, 0:131], np.ones((128, 2), np.float32)],
                         axis=1)                                   # [128, 133]
    # 3-band concatenated masks (bands are processed in groups of three
    # sharing one psum tile / one exp / one mask multiply)
    cg = np.concatenate([cI, cI, cI], axis=1)                      # [128, 408]
    cg15 = np.concatenate([cI, cI, c15], axis=1)                   # [128, 405]
    bf = ml_dtypes.bfloat16
    return c0.astype(bf), cg.astype(bf), cg15.astype(bf)


def _host_idx_w(random_indices):
    """Gather indices (t, m, i order) and dedup weights w [128, 48]."""
    import ml_dtypes

    ri = np.asarray(random_indices).astype(np.int64)   # [S, NR]
    n = np.arange(NIDX)
    t_of = n // (NR * 128)
    m_of = (n // 128) % NR
    p_of = n % 128
    i_of = t_of * 128 + p_of
    j_of = ri[i_of, m_of]
    a16 = np.zeros((16, NIDX // 16), dtype=np.int16)
    a16[n % 16, n // 16] = j_of.astype(np.int16)
    gidx = np.tile(a16, (8, 1))                        # [128, NIDX//16]

    # w[p, (t, m)]: 0 if structurally covered / global row / duplicate
    w = np.ones((128, NCH), dtype=np.float32)
    for t in range(NT):
        for m in range(NR):
            c = t * NR + m
            for p in range(128):
                i = t * 128 + p
                r = ri[i, m]
                if i < NG or r < NG or abs(i - r) <= WIN:
                    w[p, c] = 0.0
                elif any(ri[i, mm] == r for mm in range(m)):
                    w[p, c] = 0.0
    return gidx, w.astype(ml_dtypes.bfloat16)


def make_in_maps(inputs):
    """Full inputs -> list of 8 per-core input dicts."""
    import ml_dtypes
    bf = ml_dtypes.bfloat16

    x = np.asarray(inputs["x"], dtype=np.float32)
    ri = np.asarray(inputs["random_indices"])
    q_w = np.asarray(inputs["q_w"], dtype=np.float32) * INV_SQRT_HD
    k_w = np.asarray(inputs["k_w"], dtype=np.float32)
    v_w = np.asarray(inputs["v_w"], dtype=np.float32)
    o_w = np.asarray(inputs["o_w"], dtype=np.float32)
    q_b = np.asarray(inputs["q_b"], dtype=np.float32) * INV_SQRT_HD
    k_b = np.asarray(inputs["k_b"], dtype=np.float32)
    v_b = np.asarray(inputs["v_b"], dtype=np.float32)

    # x blocks: xprep[p, rb, dc, rl] = xT[dc*128+p, rb*128+rl]
    xT = np.ascontiguousarray(x.reshape(R, D).T)                  # [D, R]
    xprep = np.ascontiguousarray(
        xT.reshape(8, 128, NRB, 128).transpose(1, 2, 0, 3)).astype(bf)

    mc0, mcI, mc15 = _host_masks()
    gidx, wrand = _host_idx_w(ri)

    in_maps = []
    for c in range(N_CORES):
        sl = slice(HD2 * c, HD2 * (c + 1))
        wqkv = np.concatenate(
            [q_w[sl, :].T, k_w[sl, :].T, v_w[sl, :].T], axis=1)   # [D, 384]
        wqkv = np.ascontiguousarray(wqkv.reshape(8, 128, 384)).astype(bf)
        bias = np.concatenate([q_b[sl], k_b[sl], v_b[sl]])        # [384]
        biasb = np.ascontiguousarray(
            np.broadcast_to(bias[None, :], (128, 384))).copy()
        in_maps.append({
            "xprep": xprep,
            "wqkv": wqkv,
            "biasb": biasb,
            "mc0": mc0, "mcI": mcI, "mc15": mc15,
            "wrand": wrand,
            "gidx": gidx,
            "wo2": np.ascontiguousarray(o_w[:, sl].T).astype(bf),  # [128, D]
        })
    return in_maps


# ---------------------------------------------------------------- device IR
def build_kernel(bias_zero=True):
    import os
    import concourse.tile as tile
    from concourse import bacc, mybir

    nc = bacc.Bacc("TRN2", target_bir_lowering=False, debug=False,
                   num_swdge_queues=int(os.environ.get("K_GQ", "4")))
    f32 = mybir.dt.float32
    bf16 = mybir.dt.bfloat16
    i16 = mybir.dt.int16

    t_ = dict(
        xprep=nc.dram_tensor("xprep", [128, NRB, 8, 128], bf16,
                             kind="ExternalInput").ap(),
        wqkv=nc.dram_tensor("wqkv", [8, 128, 384], bf16,
                            kind="ExternalInput").ap(),
        biasb=nc.dram_tensor("biasb", [128, 384], f32,
                             kind="ExternalInput").ap(),
        mc0=nc.dram_tensor("mc0", [128, 2048], bf16,
                           kind="ExternalInput").ap(),
        mcI=nc.dram_tensor("mcI", [128, 408], bf16,
                           kind="ExternalInput").ap(),
        mc15=nc.dram_tensor("mc15", [128, 405], bf16,
                            kind="ExternalInput").ap(),
        wrand=nc.dram_tensor("wrand", [128, NCH], bf16,
                             kind="ExternalInput").ap(),
        gidx=nc.dram_tensor("gidx", [128, NIDX // 16], i16,
                            kind="ExternalInput").ap(),
        wo2=nc.dram_tensor("wo2", [HD2, D], bf16, kind="ExternalInput").ap(),
        out=nc.dram_tensor("out_part", [R, D], bf16,
                           kind="ExternalOutput").ap(),
        kv_stage=nc.dram_tensor("kv_stage", [S, KVROW], bf16).ap(),
    )
    if os.environ.get("K_DEBUG", "0") == "1":
        for nm, shp in (("dbg_qrows", [128, NRB, 128]),
                        ("dbg_kvrows", [128, NRB, 256]),
                        ("dbg_qT", [128, R]), ("dbg_kT", [128, R]),
                        ("dbg_ksel0", [128, NCH, 256]),
                        ("dbg_ksel1", [128, NCH, 256]),
                        ("dbg_pj", [128, 4096]),
                        ("dbg_wsumf", [128, NT, HD + 1]),
                        ("dbg_hout0", [HD, R]), ("dbg_hout1", [HD, R])):
            dt = bf16
            t_[nm] = nc.dram_tensor(nm, shp, dt, kind="ExternalOutput").ap()

    with tile.TileContext(nc) as tc:
        _build_tc(nc, tc, t_, bias_zero)
    nc.compile()
    return nc


def _build_tc(nc, tc, t_, bias_zero):
    import os
    from contextlib import ExitStack

    import concourse.bass as bass
    from concourse import masks as cmasks, mybir

    GQ = int(os.environ.get("K_GQ", "4"))
    XQ_ACT = os.environ.get("K_XQ", "act") == "act"
    NORM2P = os.environ.get("K_NORM2P", "1") == "1"   # ctx*bc both-psum mult
    DEBUG = os.environ.get("K_DEBUG", "0") == "1"

    def dbg(name, src_ap):
        if DEBUG and name in t_:
            nc.sync.dma_start(t_[name], src_ap)

    f32 = mybir.dt.float32
    bf16 = mybir.dt.bfloat16
    EXP = mybir.ActivationFunctionType.Exp
    COPY = mybir.ActivationFunctionType.Copy
    MULT = mybir.AluOpType.mult
    ADD = mybir.AluOpType.add
    X = mybir.AxisListType.X
    ts = bass.ts

    with ExitStack() as ctx:
        const = ctx.enter_context(tc.tile_pool(name="const", bufs=1))
        persist = ctx.enter_context(tc.tile_pool(name="persist", bufs=1))

        # ---- constants, spread over HWDGE queues so wqkv lands first
        ident = const.tile([128, 128], bf16)
        cmasks.make_identity(nc, ident[:])
        ones_t = const.tile([HD + 1, HD], bf16)
        nc.vector.memset(ones_t[:], 1.0)

        # wqkv first on the SP queue, split per dc so the first stage A
        # matmul only waits for its own contraction slice
        wqkv_sb = const.tile([128, 8, 384], bf16)
        for dc in range(8):
            nc.sync.dma_start(wqkv_sb[:, dc, :], t_["wqkv"][dc, :, :])
        biasb_sb = None
        if not bias_zero:
            biasb_sb = const.tile([128, 384], f32)
            nc.sync.dma_start(biasb_sb[:], t_["biasb"])
        # gather tables ride the Pool queue (ahead of the SWDGE gathers)
        gidx_sb = const.tile([128, NIDX // 16], mybir.dt.int16)
        nc.gpsimd.dma_start(gidx_sb[:], t_["gidx"])
        wrand_sb = const.tile([128, NCH], bf16)
        nc.gpsimd.dma_start(wrand_sb[:], t_["wrand"])
        mask_sb = {
            0: const.tile([128, 2048], bf16, name="mc0"),
            2: const.tile([128, 408], bf16, name="mcI"),
            3: const.tile([128, 405], bf16, name="mc15"),
        }
        for k, nm in ((0, "mc0"), (2, "mcI"), (3, "mc15")):
            nc.sync.dma_start(mask_sb[k][:], t_[nm])
        wo2_sb = const.tile([HD2, D], bf16)
        nc.sync.dma_start(wo2_sb[:], t_["wo2"])

        # ---- persistent activations
        qrows = persist.tile([128, NRB, 128], bf16)        # row-major q
        qT = persist.tile([128, R], bf16)
        kT = persist.tile([128, R], bf16)
        vaug = [[persist.tile([128, NT, HD + 1], bf16, name=f"vaug{b}{h}")
                 for h in range(HPC)] for b in range(B)]
        # h0's context lives in hout2[0:64]; h1 normalizes into a base-0
        # scratch then DMA-copies across partitions into hout2[64:128]
        hout2 = persist.tile([128, R], bf16)
        hout1 = persist.tile([HD, R], bf16)
        kselp = ctx.enter_context(tc.tile_pool(name="kselp", bufs=1))
        ksel = kselp.tile([128, NCH, KVROW], bf16)         # gathered KV rows

        # ---- stage A: fused row-major QKV projection (pairs of row blocks)
        with tc.tile_pool(name="kvrows", bufs=1) as kvpool:
            kvrows = kvpool.tile([128, NRB, 256], bf16)    # row-major k|v
            with tc.tile_pool(name="xstream", bufs=6) as xpool, \
                    tc.tile_pool(name="apsum", bufs=3, space="PSUM") as apsum:
                for pr in range(NRB // 2):
                    xt = xpool.tile([128, 2, 8, 128], bf16, tag="xt")
                    xeng = nc.scalar if XQ_ACT else nc.sync
                    xeng.dma_start(
                        xt[:], t_["xprep"][:, 2 * pr:2 * pr + 2, :, :])
                    # pad to 512/f32 so each j's output stays in one bank
                    ps = apsum.tile([128, 2, 512], f32, tag="ps")
                    for j in range(2):
                        for dc in range(8):
                            nc.tensor.matmul(ps[:, j, 0:384],
                                             xt[:, j, dc, :],
                                             wqkv_sb[:, dc, :],
                                             start=(dc == 0), stop=(dc == 7))
                    rb = 2 * pr
                    if bias_zero:
                        if pr % 2 == 0:
                            nc.scalar.activation(qrows[:, rb:rb + 2, :],
                                                 ps[:, :, 0:128], COPY)
                            nc.scalar.activation(kvrows[:, rb:rb + 2, :],
                                                 ps[:, :, 128:384], COPY)
                        else:
                            nc.vector.tensor_copy(qrows[:, rb:rb + 2, :],
                                                  ps[:, :, 0:128])
                            nc.vector.tensor_copy(kvrows[:, rb:rb + 2, :],
                                                  ps[:, :, 128:384])
                    else:
                        nc.vector.tensor_tensor(
                            qrows[:, rb:rb + 2, :], ps[:, :, 0:128],
                            biasb_sb[:, 0:128].unsqueeze(1)
                            .broadcast_to([128, 2, 128]), op=ADD)
                        nc.vector.tensor_tensor(
                            kvrows[:, rb:rb + 2, :], ps[:, :, 128:384],
                            biasb_sb[:, 128:384].unsqueeze(1)
                            .broadcast_to([128, 2, 256]), op=ADD)
                    b = rb // NT
                    r0 = (rb % NT) * 128
                    nc.sync.dma_start(
                        t_["kv_stage"][r0:r0 + 256, b * 256:(b + 1) * 256]
                        .rearrange("(r p) c -> p r c", p=128),
                        kvrows[:, rb:rb + 2, :])

            # ---- gathers: 6 row-mode SWDGE calls across GQ queues
            for u in range(NIDX // 1024):
                nc.gpsimd.dma_gather(
                    ksel[:, u * 8:(u + 1) * 8, :], t_["kv_stage"],
                    gidx_sb[:, u * 64:(u + 1) * 64],
                    1024, 1024, KVROW, transpose=False, queue_num=u % GQ)

            # ---- qT / kT transposes
            with tc.tile_pool(name="tpsum", bufs=4, space="PSUM") as tpsum:
                for rb in range(NRB):
                    ptq = tpsum.tile([128, 128], bf16, tag="tp")
                    nc.tensor.transpose(ptq[:], qrows[:, rb, :], ident[:])
                    ptk = tpsum.tile([128, 128], bf16, tag="tp")
                    nc.tensor.transpose(ptk[:], kvrows[:, rb, 0:128],
                                        ident[:])
                    if rb % 2 == 0:
                        nc.scalar.activation(qT[:, ts(rb, 128)], ptq[:],
                                             COPY)
                        nc.vector.tensor_copy(kT[:, ts(rb, 128)], ptk[:])
                    else:
                        nc.vector.tensor_copy(qT[:, ts(rb, 128)], ptq[:])
                        nc.scalar.activation(kT[:, ts(rb, 128)], ptk[:],
                                             COPY)

            # ---- vaug: V rows + ones rider
            for b in range(B):
                for h in range(HPC):
                    nc.vector.tensor_copy(
                        vaug[b][h][:, :, 0:HD],
                        kvrows[:, b * NT:(b + 1) * NT,
                               128 + HD * h:128 + HD * (h + 1)])
                    nc.vector.memset(vaug[b][h][:, :, HD:HD + 1], 1.0)

            dbg("dbg_qrows", qrows[:])
            dbg("dbg_kvrows", kvrows[:])
            dbg("dbg_qT", qT[:])
            dbg("dbg_kT", kT[:])
            dbg("dbg_ksel0", ksel[:, :, 0:256])
            dbg("dbg_ksel1", ksel[:, :, 256:512])

        # ---- attention
        with tc.tile_pool(name="pj", bufs=4) as pjpool, \
                tc.tile_pool(name="rbig", bufs=4) as rbig, \
                tc.tile_pool(name="rsm", bufs=4) as rsm, \
                tc.tile_pool(name="wsb", bufs=4) as wpool, \
                tc.tile_pool(name="dn2", bufs=2) as dn2, \
                tc.tile_pool(name="dn4", bufs=4) as dn4, \
                tc.tile_pool(name="osb", bufs=3) as opool, \
                tc.tile_pool(name="ctx", bufs=1, space="PSUM") as ctxpsum, \
                tc.tile_pool(name="s512", bufs=3, space="PSUM") as spsum, \
                tc.tile_pool(name="bc", bufs=1, space="PSUM") as bcpsum:

            pj_t = {}
            coef_t = {}
            wsumb_t = {}
            deferred_masks = []

            def qk_phase(b, h, mask_now=True):
                hs = slice(HD * h, HD * (h + 1))
                pj = pjpool.tile([128, 4096], bf16, tag="pj",
                                 name=f"pj{b}{h}")
                pj_t[(b, h)] = pj

                def mask_mult(lo, tot, kind, seg=None):
                    msk = (mask_sb[0][:, ts(seg, 512)] if kind == 0
                           else mask_sb[kind][:, 0:tot])
                    op = (nc.vector.tensor_tensor,
                          pj[:, lo:lo + tot], msk)
                    if mask_now:
                        op[0](op[1], op[1], op[2], op=MULT)
                    else:
                        deferred_masks.append(op)

                # j-block 0 serves every i column
                for seg in range(4):
                    psc = spsum.tile([128, 512], f32, tag="psc")
                    nc.tensor.matmul(
                        psc[:], kT[hs, b * S:b * S + 128],
                        qT[hs, b * S + seg * 512:b * S + (seg + 1) * 512],
                        start=True, stop=True)
                    nc.scalar.activation(
                        pj[:, seg * 512:(seg + 1) * 512], psc[:], EXP)
                    mask_mult(seg * 512, 512, 0, seg)
                col = 2048
                # bands in groups of three sharing one psum tile, one exp
                # and one mask multiply
                for g in range(5):
                    psc = spsum.tile([128, 512], f32, tag="psc")
                    off = 0
                    for jb in range(3 * g + 1, 3 * g + 4):
                        i0, w, extra = _serve(jb)
                        lhsT = kT[hs, b * S + jb * 128:
                                  b * S + (jb + 1) * 128]
                        nc.tensor.matmul(psc[:, off:off + w], lhsT,
                                         qT[hs, b * S + i0:b * S + i0 + w],
                                         start=(jb == 3 * g + 1), stop=False)
                        # start=False: the group's first matmul bank-clear
                        # already reset has_written for this psum tile
                        nc.tensor.matmul(
                            psc[:, off + w:off + w + 2], lhsT,
                            qT[hs, b * S:b * S + 2],
                            start=False, stop=(jb == 3 * g + 3))
                        off += w + 2
                    nc.scalar.activation(pj[:, col:col + off],
                                         psc[:, 0:off], EXP)
                    mask_mult(col, off, 3 if g == 4 else 2)
                    col += off
                if (b, h) == (0, 0):
                    dbg("dbg_pj", pj[:])

            def flush_masks():
                for fn, ap, msk in deferred_masks:
                    fn(ap, ap, msk, op=MULT)
                deferred_masks.clear()

            def rand_a(b, h, eng):
                # random-key scores: multiply on eng, DVE reduce, Scalar exp
                kv_k = ksel[:, :, b * 256 + HD * h:b * 256 + HD * (h + 1)]
                kv_k = kv_k.rearrange("p (t m) c -> p t m c", t=NT)
                qrep = qrows[:, b * NT:(b + 1) * NT, HD * h:HD * (h + 1)]
                qrep = qrep.unsqueeze(2).broadcast_to([128, NT, NR, HD])
                dotp = rbig.tile([128, NT, NR, HD], bf16, tag="big",
                                 name=f"dotp{b}{h}")
                eng.tensor_tensor(dotp[:], kv_k, qrep, op=MULT)
                sval = rsm.tile([128, NCH], f32, tag="sval")
                nc.vector.tensor_reduce(
                    sval[:].rearrange("p (t m) -> p t m", t=NT), dotp[:],
                    axis=X, op=ADD)
                coef = rsm.tile([128, NCH], bf16, tag="coef",
                                name=f"coef{b}{h}")
                coef_t[(b, h)] = coef
                nc.scalar.activation(coef[:], sval[:], EXP)
                nc.vector.tensor_tensor(coef[:], coef[:], wrand_sb[:],
                                        op=MULT)

            def rand_b(b, h, eng):
                # weighted V rows: multiply on eng, DVE reduces straight to
                # bf16 (summing <=3 O(1) terms — fp32 accumulate unneeded)
                coef = coef_t[(b, h)]
                kv_v = ksel[:, :, b * 256 + 128 + HD * h:
                            b * 256 + 128 + HD * (h + 1)]
                kv_v = kv_v.rearrange("p (t m) c -> p t m c", t=NT)
                crep = coef[:].rearrange("p (t m) -> p t m", t=NT)
                crep = crep.unsqueeze(3).broadcast_to([128, NT, NR, HD])
                dotv = rbig.tile([128, NT, NR, HD], bf16, tag="big",
                                 name=f"dotv{b}{h}")
                eng.tensor_tensor(dotv[:], kv_v, crep, op=MULT)
                wsumb = wpool.tile([128, NT, HD + 1], bf16, tag="wsumb",
                                   name=f"wsumb{b}{h}")
                wsumb_t[(b, h)] = wsumb
                with nc.allow_low_precision("3-term O(1) rand sums"):
                    nc.vector.tensor_reduce(
                        wsumb[:, :, 0:HD],
                        dotv[:].rearrange("p t m c -> p t c m"),
                        axis=X, op=ADD)
                    nc.vector.tensor_reduce(
                        wsumb[:, :, HD:HD + 1],
                        coef[:].rearrange("p (t m) -> p t m", t=NT),
                        axis=X, op=ADD)
                if (b, h) == (0, 0):
                    dbg("dbg_wsumf", wsumb[:])

            def pv_phase(b, h):
                pj = pj_t[(b, h)]
                wsumb = wsumb_t[(b, h)]
                ctxp = ctxpsum.tile([128, 2048], f32, tag="ctx")
                col = 0
                for jb in range(NT):
                    i0, w, extra = _serve(jb)
                    lhsT = vaug[b][h][:, jb, :]
                    if jb == 0:
                        for seg in range(4):
                            nc.tensor.matmul(
                                ctxp[0:HD + 1, ts(seg, 512)], lhsT,
                                pj[:, ts(seg, 512)], start=True, stop=False)
                        col = 2048
                        continue
                    # split at 512-col PSUM bank boundaries
                    a = i0
                    while a < i0 + w:
                        e = min(i0 + w, (a // 512 + 1) * 512)
                        nc.tensor.matmul(
                            ctxp[0:HD + 1, a:e], lhsT,
                            pj[:, col + (a - i0):col + (e - i0)],
                            start=False, stop=False)
                        a = e
                    if extra:
                        nc.tensor.matmul(ctxp[0:HD + 1, 0:2], lhsT,
                                         pj[:, col + w:col + w + 2],
                                         start=False, stop=False)
                    col += w + (2 if extra else 0)
                # wsum merges per 512-seg; den copy as soon as a seg's
                # columns are final, so the bcast matmul never waits
                dens = []
                for sgi in range(4):
                    for t in range(4 * sgi, 4 * sgi + 4):
                        nc.tensor.matmul(ctxp[0:HD + 1, ts(t, 128)],
                                         wsumb[:, t, :], ident[:],
                                         start=False, stop=True)
                    den = dn4.tile([HD + 1, 512], bf16, tag="den")
                    nc.scalar.activation(den[HD:HD + 1, :],
                                         ctxp[HD:HD + 1, ts(sgi, 512)], COPY)
                    dens.append(den)
                dst = hout2[0:HD, :] if h == 0 else hout1[:]
                for sgi in range(4):
                    bcp = bcpsum.tile([HD, 512], f32, tag="bc")
                    nc.tensor.matmul(bcp[:], ones_t[HD:HD + 1, :],
                                     dens[sgi][HD:HD + 1, :],
                                     start=True, stop=True)
                    # approx-fast reciprocal misreads PSUM sources: stage
                    # via SBUF (GPSIMD cannot read PSUM, so Scalar does it)
                    bsb = dn2.tile([HD, 512], f32, tag="bsb")
                    nc.scalar.activation(bsb[:], bcp[:], COPY)
                    rinv = dn2.tile([HD, 512], f32, tag="rinv")
                    nc.vector.reciprocal_approx_fast(rinv[:], bsb[:])
                    nc.vector.tensor_tensor(
                        dst[:, b * S + sgi * 512:b * S + (sgi + 1) * 512],
                        ctxp[0:HD, ts(sgi, 512)], rinv[:], op=MULT)
                if h == 1:
                    # cross-partition merge (only DMA can change partitions)
                    nc.sync.dma_start(
                        hout2[HD:HD2, b * S:(b + 1) * S],
                        hout1[:, b * S:(b + 1) * S])

            def d_stage(b, scalar_only=False):
                # partial o-projection for batch b (interleaves attention)
                for rc in range(b * NT, (b + 1) * NT):
                    ob = opool.tile([128, D], bf16, tag="ob")
                    for half in range(2):
                        po = spsum.tile([128, 512], f32, tag="psc")
                        nc.tensor.matmul(po[:], hout2[:, ts(rc, 128)],
                                         wo2_sb[:, ts(half, 512)],
                                         start=True, stop=True)
                        if not scalar_only and (rc * 2 + half) % 2 == 0:
                            nc.vector.tensor_copy(ob[:, ts(half, 512)], po[:])
                        else:
                            nc.scalar.activation(ob[:, ts(half, 512)], po[:],
                                                 COPY)
                    # alternate HWDGE queues for the output stream
                    oeng = nc.sync if rc % 2 == 0 else nc.scalar
                    oeng.dma_start(t_["out"][ts(rc, 128), :], ob[:])

            # software pipeline: all four QK phases first (pj bufs=4), then
            # PVs in order.  Pair (0,0)'s whole rand chain runs on DVE
            # (lowest latency after the gathers land); the other pairs'
            # multiplies go to Pool.  Masks for the second batch are
            # deferred past pair (0,0)'s chain so the DVE FIFO serves the
            # critical chain first.
            qk_phase(0, 0)
            qk_phase(0, 1)
            qk_phase(1, 0, mask_now=False)
            qk_phase(1, 1, mask_now=False)
            rand_a(0, 0, nc.vector)
            rand_b(0, 0, nc.gpsimd)
            flush_masks()
            pv_phase(0, 0)
            rand_a(0, 1, nc.gpsimd)
            rand_a(1, 0, nc.gpsimd)
            rand_a(1, 1, nc.gpsimd)
            rand_b(0, 1, nc.gpsimd)
            pv_phase(0, 1)
            rand_b(1, 0, nc.gpsimd)
            d_stage(0, scalar_only=True)
            rand_b(1, 1, nc.gpsimd)
            pv_phase(1, 0)
            pv_phase(1, 1)
            dbg("dbg_hout0", hout2[0:HD, :])
            dbg("dbg_hout1", hout1[:])

        # ---- stage D batch 1 (tail): wide psum tiles, single wide copies
        with tc.tile_pool(name="osb2", bufs=4) as opool2, \
                tc.tile_pool(name="opsum", bufs=2, space="PSUM") as opsum:
            for rc in range(NT, NRB):
                ob = opool2.tile([128, D], bf16, tag="ob")
                po = opsum.tile([128, 2, 512], f32, tag="po")
                for half in range(2):
                    nc.tensor.matmul(po[:, half, :], hout2[:, ts(rc, 128)],
                                     wo2_sb[:, ts(half, 512)],
                                     start=True, stop=True)
                if rc % 2 == 0:
                    nc.vector.tensor_copy(ob[:], po[:])
                else:
                    nc.scalar.activation(ob[:], po[:], COPY)
                oeng = nc.sync if rc % 2 == 0 else nc.scalar
                oeng.dma_start(t_["out"][ts(rc, 128), :], ob[:])


# ---------------------------------------------------------------- execution
_NC_CACHE = {}


def _get_nc(bias_zero=True):
    if bias_zero not in _NC_CACHE:
        _NC_CACHE[bias_zero] = build_kernel(bias_zero)
    return _NC_CACHE[bias_zero]


def _install_axon_trace_shim():
    import sys
    import types

    if "antenv.axon_hooks" in sys.modules:
        return
    mod = types.ModuleType("antenv.axon_hooks")
    mod._hook = None
    mod.set_axon_ntff_profile_hook = lambda h: setattr(mod, "_hook", h)
    mod.get_axon_ntff_profile_hook = lambda: mod._hook
    sys.modules["antenv.axon_hooks"] = mod
    try:
        import antenv
        antenv.axon_hooks = mod
        from trn_agent_boot.trn_boot import _ntff_profile_via_ctypes
        mod._hook = _ntff_profile_via_ctypes("/opt/axon/libaxon_pjrt.so")
    except Exception:
        pass


def run_on_hw(in_maps, trace=False, trace_kwargs=None, bias_zero=True):
    """Compile+run on the 8 cores; returns (results, BassKernelResults)."""
    _install_axon_trace_shim()
    from concourse import bass_utils
    bass_utils.upload_artifacts = lambda tmpdir: f"local:{tmpdir}"

    nc = _get_nc(bias_zero)
    res = bass_utils.run_bass_kernel_spmd(
        nc, in_maps, core_ids=list(range(N_CORES)), trace=trace,
        trace_kwargs=trace_kwargs or {})
    return res.results, res


def _bias_zero(inputs):
    return all(
        not np.any(np.asarray(inputs[k], dtype=np.float32))
        for k in ("q_b", "k_b", "v_b"))


def kernel(**inputs):
    in_maps = make_in_maps(inputs)
    results, _ = run_on_hw(in_maps, trace=False, bias_zero=_bias_zero(inputs))
    out = np.zeros((R, D), dtype=np.float32)
    for c in range(N_CORES):
        out += np.asarray(results[c]["out_part"], dtype=np.float32)
    out += np.asarray(inputs["o_b"], dtype=np.float32)[None, :]
    return out.reshape(B, S, D)
